# revision 38
# baseline (speedup 1.0000x reference)
"""DistSageConv on 8 TRN2 NeuronCores (Bass/Tile) — v2 overlapped.

Reference computation:
    out  = x @ W1.T + b1                                  # [n_src, 128]
    out1 = segment_sum(out[src_ids], dst_ids, n_dst)      # [n_dst, 128]
    out5 = x[:n_dst] @ W2.T + b2
    return out5 + out1

Distribution: src nodes sharded across 8 cores; each core projects its own
src shard (bf16) into a local bf16 table in DRAM, gathers its own-src edges'
rows with SWDGE dma_gather, segment-reduces them with one-hot matmuls on the
PE (dst grouped into 128-row blocks, XOR-permuted owner-major column order),
then partial aggregates are exchanged:

  - columns are processed in order [grp4/5 | grp6/7 | grp2/3 | grp1 | grp0]
    (far-owner halves first) so the two far-half sends (decl 6) fire DURING
    phase 2 and their D2D wire time hides under the remaining gather work;
  - the on-die remainder is exchanged with three DIRECT per-owner sends
    (decl 1/2/3), overlapped with the tail of phase 2 via notify semaphores
    that let peers reuse the RECV buffers as soon as the far-half adds have
    consumed them.

Owners fuse x[:n_dst] @ W2.T (+bias, +deg*b1 via an augmented K=258 matmul)
and write their 5120-row output slab.  All x-side inputs are fed in bf16
(PSUM accumulation stays f32).
"""
import sys
sys.path.insert(0, "/opt/trn_rl_repo")

import numpy as np
import ml_dtypes

import os
import concourse.bacc as bacc
import concourse.bass as bass
import concourse.mybir as mybir
import concourse.tile as tile
from concourse import library_config
from concourse.bass_utils import run_bass_kernel_spmd
from concourse.tile import add_dep_helper

# ---------------- problem constants (hardcoded per contract) --------------
P = 8                      # cores
N_SRC = 100000
N_DST = 40000
N_EDGES = 640000
INF = 256                  # in_feats
OUTF = 128                 # out_feats
SRC_SH = N_SRC // P        # 12500 src rows per core
SRC_PAD = 12800            # padded table rows (25 x 512)
NBLK = 320                 # padded dst blocks of 128 (40960 dst rows)
BPC = NBLK // P            # 40 blocks (columns) owned per core
DST_PAD = NBLK * 128       # 40960
CPC = 2048                 # gather chunk edges
TPC = CPC // 128           # 16 tiles per chunk
NQ = 4                     # SWDGE queues for gathers (round-robin, overlap)
RQ = 0                     # SWDGE queue for remote sends/notifies (all fired
                           # after the gathers have fully drained, as in the
                           # baseline: mid-phase2 remote sends stall for
                           # milliseconds — the interleaved library reloads
                           # appear to wreck in-flight SWDGE ring state)

F32 = mybir.dt.float32
BF16 = mybir.dt.bfloat16
I16 = mybir.dt.int16

# column processing order: far-half groups first, own group (0) last
COL_ORDER = (list(range(160, 240)) + list(range(240, 320)) +
             list(range(80, 160)) + list(range(40, 80)) + list(range(0, 40)))

_CACHE = {}


# ============================ host-side prep ==============================

def _wrap_idxs(idx):
    """[n] int16 -> [128, n//16] wrapped in 16 partitions, replicated x8."""
    n = len(idx)
    w = np.zeros((128, n // 16), dtype=np.int16)
    for p in range(16):
        w[p, :] = idx[p::16]
    for r in range(1, 8):
        w[16 * r:16 * r + 16, :] = w[:16, :]
    return w


def _host_prep(x, W1, b1, W2, b2, src_ids, dst_ids):
    """Build per-core input arrays + the static tile->column schedule."""
    x = np.asarray(x, np.float32)
    W1 = np.asarray(W1, np.float32)
    W2 = np.asarray(W2, np.float32)
    b1 = np.asarray(b1, np.float32).reshape(-1)
    b2 = np.asarray(b2, np.float32).reshape(-1)
    src_ids = np.asarray(src_ids, np.int64)
    dst_ids = np.asarray(dst_ids, np.int64)

    owner = src_ids // SRC_SH                       # edge -> src-owner core
    blk = dst_ids // 128                            # edge -> dst block
    deg_full = np.bincount(dst_ids, minlength=DST_PAD).astype(np.float32)

    # processing position of each column
    pos_of_col = np.zeros(NBLK, dtype=np.int64)
    for i, j in enumerate(COL_ORDER):
        pos_of_col[j] = i

    # per-(core, column) counts;  column j on core c holds block beta_c(j)
    per_core = []
    for c in range(P):
        m = owner == c
        e_src = (src_ids[m] - c * SRC_SH).astype(np.int64)
        e_dst = dst_ids[m]
        e_blk = blk[m]
        col = ((e_blk // BPC) ^ c) * BPC + (e_blk % BPC)
        order = np.argsort(pos_of_col[col], kind="stable")
        per_core.append((e_src[order], e_dst[order], col[order]))

    counts = np.zeros((P, NBLK), dtype=np.int64)
    for c in range(P):
        counts[c] = np.bincount(per_core[c][2], minlength=NBLK)
    t_col = np.maximum(1, (counts.max(axis=0) + 127) // 128)   # tiles per col
    nt_tot = int(t_col.sum())
    spc = NQ * TPC             # tiles per super-chunk (NQ queues bunched)
    nt_pad = ((nt_tot + spc - 1) // spc) * spc
    nchunk = nt_pad // TPC
    # col_base in PROCESSING order
    col_base = np.zeros(NBLK + 1, dtype=np.int64)
    acc = 0
    for j in COL_ORDER:
        col_base[j] = acc
        acc += t_col[j]

    # static schedule: per tile -> col ;  col -1 => pad tile
    tile_col = np.full(nt_pad, -1, dtype=np.int64)
    for j in range(NBLK):
        tile_col[col_base[j]:col_base[j] + t_col[j]] = j

    # stream positions where the exchange G-blocks complete
    g0_end = int(sum(t_col[j] for j in range(160, 240)))
    g1_end = g0_end + int(sum(t_col[j] for j in range(240, 320)))

    in_maps = []
    iota = np.broadcast_to(np.arange(128, dtype=np.float32), (128, 128))
    iota = np.ascontiguousarray(iota.astype(ml_dtypes.bfloat16))
    W1T = np.ascontiguousarray(W1.T.astype(ml_dtypes.bfloat16))   # [256, 128]
    W2T_aug = np.concatenate([W2.T, b1[None, :], b2[None, :]], axis=0)
    W2T_aug = np.ascontiguousarray(W2T_aug.astype(ml_dtypes.bfloat16))

    for c in range(P):
        e_src, e_dst, e_col = per_core[c]
        # scatter edges into padded per-column tile runs
        # pad slots must NOT all hit one table row (HBM hot-bank) —
        # spread them across the table; dst_arr=-1 keeps onehot columns zero.
        idx_arr = (np.arange(nt_pad * 128) % SRC_PAD).astype(np.int16)
        dst_arr = np.full(nt_pad * 128, -1.0, dtype=np.float32)
        cc = np.bincount(e_col, minlength=NBLK)
        cstart = np.zeros(NBLK + 1, dtype=np.int64)
        cstart[1:] = np.cumsum(cc)
        # position of each edge inside its column run (edges sorted by
        # processing position, stable → within col they are consecutive)
        pos_in_col = np.zeros(len(e_col), dtype=np.int64)
        run_start = np.zeros(NBLK, dtype=np.int64)
        # edges of column j occupy a consecutive run; compute run starts in
        # processing order
        acc2 = 0
        for j in COL_ORDER:
            run_start[j] = acc2
            acc2 += cc[j]
        pos_in_col = np.arange(len(e_col)) - run_start[e_col]
        gpos = col_base[e_col] * 128 + pos_in_col
        idx_arr[gpos] = e_src.astype(np.int16)
        e_blk_of_col = ((e_col // BPC) ^ c) * BPC + (e_col % BPC)
        dst_arr[gpos] = (e_dst - e_blk_of_col * 128).astype(np.float32)

        idx_dram = np.zeros((nchunk, 128, CPC // 16), dtype=np.int16)
        for ch in range(nchunk):
            idx_dram[ch] = _wrap_idxs(idx_arr[ch * CPC:(ch + 1) * CPC])
        dst_dram = dst_arr.reshape(nchunk, TPC, 128).transpose(2, 0, 1)
        dst_dram = np.ascontiguousarray(
            dst_dram.reshape(128, nchunk * TPC).astype(ml_dtypes.bfloat16))

        xT = np.zeros((INF, SRC_PAD), dtype=ml_dtypes.bfloat16)
        xT[:, :SRC_SH] = x[c * SRC_SH:(c + 1) * SRC_SH].T.astype(
            ml_dtypes.bfloat16)
        lo, hi = c * 5120, min((c + 1) * 5120, N_DST)
        xdT = np.zeros((INF + 2, 5120), dtype=np.float32)
        if hi > lo:
            xdT[:INF, :hi - lo] = x[lo:hi].T
        xdT[INF, :] = deg_full[c * 5120:(c + 1) * 5120]
        xdT[INF + 1, :] = 1.0
        xdT = xdT.astype(ml_dtypes.bfloat16)

        in_maps.append({
            "xT": np.ascontiguousarray(xT),
            "xdT": np.ascontiguousarray(xdT),
            "W1T": W1T,
            "W2Ta": W2T_aug,
            "iota": iota,
            "idx": idx_dram,
            "dstloc": dst_dram,
        })

    sched = {"t_col": t_col, "nchunk": nchunk, "tile_col": tile_col,
             "col_base": col_base, "g0_end": g0_end, "g1_end": g1_end}
    return in_maps, sched


# ============================ device program ==============================

def _build(sched):
    t_col = sched["t_col"]
    nchunk = sched["nchunk"]
    tile_col = sched["tile_col"]
    col_base = sched["col_base"]
    nt_pad = nchunk * TPC
    nsc = nchunk // NQ          # super-chunks

    # super-chunk indices after which the R0 / R1 remote sections run.
    # Desc-gen is consumption-throttled (gather pool depth), so it lags the
    # flushes; fire the sends shortly after the G-block tiles are issued.
    LEAD = NQ * TPC
    sc_r0 = min(nsc - 1, (sched["g0_end"] + LEAD) // (NQ * TPC) + 1)
    sc_r1 = min(nsc - 1, max(sc_r0 + 1,
                             (sched["g1_end"] + LEAD) // (NQ * TPC) + 1))

    nc = bacc.Bacc("TRN2", target_bir_lowering=False, debug=False,
                   num_devices=P, num_swdge_queues=NQ)

    xT_d = nc.dram_tensor("xT", [INF, SRC_PAD], BF16, kind="ExternalInput")
    xdT_d = nc.dram_tensor("xdT", [INF + 2, 5120], BF16, kind="ExternalInput")
    W1T_d = nc.dram_tensor("W1T", [INF, OUTF], BF16, kind="ExternalInput")
    W2Ta_d = nc.dram_tensor("W2Ta", [INF + 2, OUTF], BF16, kind="ExternalInput")
    iota_d = nc.dram_tensor("iota", [128, 128], BF16, kind="ExternalInput")
    idx_d = nc.dram_tensor("idx", [nchunk, 128, CPC // 16], I16, kind="ExternalInput")
    dst_d = nc.dram_tensor("dstloc", [128, nchunk * TPC], BF16, kind="ExternalInput")
    out_d = nc.dram_tensor("out", [5120, OUTF], F32, kind="ExternalOutput")
    tab_d = nc.dram_tensor("tab", [SRC_PAD, OUTF], BF16, kind="Internal")

    # persistent SBUF: partial slab + exchange recv buffers
    PART = nc.alloc_sbuf_tensor("part", [128, NBLK * 128], BF16)
    RECV = nc.alloc_sbuf_tensor("recv", [128, (NBLK // 4) * 128], BF16)
    RECV2 = nc.alloc_sbuf_tensor("recv2", [128, (NBLK // 4) * 128], BF16)
    OUT5 = nc.alloc_sbuf_tensor("out5", [128, BPC * 128], BF16)

    rsem = nc.semaphore("rsem").__enter__()      # data arrived (remote inc)
    n1sem = nc.semaphore("n1sem").__enter__()    # notify: peer freed RECV low
    n2sem = nc.semaphore("n2sem").__enter__()    # notify: peer freed RECV2 lo
    n3sem = nc.semaphore("n3sem").__enter__()    # notify: peer freed RECV2 hi
    lsem = nc.semaphore("lsem").__enter__()      # local send drained
    psem = nc.semaphore("psem").__enter__()      # desc-gen complete
    esem = nc.semaphore("esem").__enter__()      # G-block flushes done
    xsem = nc.semaphore("xsem").__enter__()      # DVE add done

    # bookkeeping for manual gpsimd sections
    state = {"npr": 0}

    with tile.TileContext(nc) as tc:
        nc.gpsimd.load_library(library_config.mlp)
        with (
            tc.tile_pool(name="consts", bufs=1) as constp,
            tc.tile_pool(name="xab", bufs=2) as xabp,
            tc.tile_pool(name="xc2", bufs=2) as xc2p,
            tc.tile_pool(name="stage", bufs=2) as stagep,
            tc.tile_pool(name="idx", bufs=8) as idxp,
            tc.tile_pool(name="gath", bufs=3) as gathp,
            tc.tile_pool(name="oh", bufs=2) as ohp,
            tc.tile_pool(name="ps", bufs=4, space="PSUM") as psp,
            tc.tile_pool(name="ps3", bufs=2, space="PSUM") as ps3p,
        ):
            # ---- constants
            iota_t = constp.tile([128, 128], BF16)
            nc.sync.dma_start(iota_t[:], iota_d[:])
            w1 = constp.tile([128, 2, OUTF], BF16)
            nc.sync.dma_start(w1[:], W1T_d[:].rearrange("(k p) f -> p k f", p=128))
            w2 = constp.tile([128, 2, OUTF], BF16)
            nc.sync.dma_start(w2[:], W2Ta_d[:INF].rearrange("(k p) f -> p k f", p=128))
            wb = constp.tile([2, OUTF], BF16)
            nc.sync.dma_start(wb[:], W2Ta_d[INF:INF + 2, :])
            DSTL = constp.tile([128, nchunk * TPC], BF16)
            nc.sync.dma_start(DSTL[:], dst_d[:])

            # ---------------- phase 1: project own src shard ----------------
            with nc.named_scope("phase1"):
                for j in range(SRC_PAD // 512):
                    a0 = xabp.tile([128, 512], BF16, tag="a0")
                    a1 = xabp.tile([128, 512], BF16, tag="a1")
                    nc.sync.dma_start(a0[:], xT_d[0:128, j * 512:(j + 1) * 512])
                    nc.sync.dma_start(a1[:], xT_d[128:256, j * 512:(j + 1) * 512])
                    ps = psp.tile([128, 512], F32, space="PSUM", tag="ps")
                    for u in range(4):
                        nc.tensor.matmul(
                            out=ps[:, u * 128:(u + 1) * 128],
                            lhsT=a0[:, u * 128:(u + 1) * 128], rhs=w1[:, 0, :],
                            start=(u == 0), stop=False)
                        nc.tensor.matmul(
                            out=ps[:, u * 128:(u + 1) * 128],
                            lhsT=a1[:, u * 128:(u + 1) * 128], rhs=w1[:, 1, :],
                            start=False, stop=(u == 3))
                    st = stagep.tile([128, 512], BF16, tag="st1")
                    nc.vector.tensor_copy(out=st[:], in_=ps[:])
                    nc.sync.dma_start(
                        tab_d[j * 512:(j + 1) * 512, :].rearrange(
                            "(u p) f -> p u f", p=128),
                        st[:].rearrange("p (u f) -> p u f", u=4))

            # -------- phase 3a: own-dst projection into OUT5 (overlaps p2) --
            with nc.named_scope("phase3a"):
                p3a_copies = []
                for grp in range(10):
                    b0 = xc2p.tile([128, 512], BF16, tag="b0")
                    b1t = xc2p.tile([128, 512], BF16, tag="b1")
                    b2t = xc2p.tile([2, 512], BF16, tag="b2")
                    nc.sync.dma_start(b0[:], xdT_d[0:128, grp * 512:(grp + 1) * 512])
                    nc.sync.dma_start(b1t[:], xdT_d[128:256, grp * 512:(grp + 1) * 512])
                    nc.sync.dma_start(b2t[:], xdT_d[256:258, grp * 512:(grp + 1) * 512])
                    ps3 = ps3p.tile([128, 512], F32, space="PSUM", tag="p3")
                    for u in range(4):
                        sl = slice(u * 128, (u + 1) * 128)
                        nc.tensor.matmul(out=ps3[:, sl], lhsT=b0[:, sl],
                                         rhs=w2[:, 0, :], start=(u == 0), stop=False)
                        nc.tensor.matmul(out=ps3[:, sl], lhsT=b1t[:, sl],
                                         rhs=w2[:, 1, :], start=False, stop=False)
                        nc.tensor.matmul(out=ps3[:, sl], lhsT=b2t[:, sl],
                                         rhs=wb[:], start=False, stop=(u == 3))
                    cp = nc.scalar.copy(
                        out=OUT5[:, grp * 512:(grp + 1) * 512], in_=ps3[:])
                    p3a_copies.append(cp)

            # ----- manual remote helpers (critical sections) ----------------
            # Remote desc-gen ops + their sem waits live inside
            # tc.tile_critical() bodies: the scheduler treats the inner basic
            # block as opaque, so the cross-core semaphore waits don't trip
            # the (single-core) deadlock checker, and per-engine emission
            # order inside the block is preserved on hardware.
            def remote_section(body):
                """Switch to remote_dma lib, run body(), switch back to mlp,
                inside one critical section."""
                with tc.tile_critical(no_gpsimd_drain=True):
                    nc.gpsimd.load_library(library_config.remote_dma)
                    for mk in body:
                        mk()
                    nc.gpsimd.load_library(library_config.mlp)

            def far_send(src_col, recv_buf):
                """4 sends of 20 cols to decl-6 peer; slots 4-7."""
                mks = []
                for i in range(4):
                    def mk(i=i):
                        rdests = [None] * 8
                        rdests[4 + i] = (0, 6)
                        return nc.gpsimd.remote_dma_broadcast(
                            out_ap=recv_buf[:, i * 20 * 128:(i + 1) * 20 * 128],
                            in_ap=PART[:, (src_col + i * 20) * 128:
                                       (src_col + (i + 1) * 20) * 128],
                            remote_sem=rsem, local_sem=lsem,
                            rdests=rdests, queue_num=RQ).then_inc(psem, 1)
                    mks.append(mk)
                state["npr"] += 4
                npr = state["npr"]
                mks.append(lambda: nc.gpsimd.wait_ge(psem, npr))
                mks.append(lambda: nc.gpsimd.trigger_dma(count=4,
                                                         queue_num=RQ))
                return mks

            def direct_send(src_col, ncols, decl, slots, recv_buf, recv_off):
                """Send split across len(slots) engine pairs."""
                sub = ncols // len(slots)
                mks = []
                for i, slot in enumerate(slots):
                    def mk(i=i, slot=slot):
                        rdests = [None] * 8
                        rdests[slot] = (0, decl)
                        return nc.gpsimd.remote_dma_broadcast(
                            out_ap=recv_buf[:, (recv_off + i * sub) * 128:
                                            (recv_off + (i + 1) * sub) * 128],
                            in_ap=PART[:, (src_col + i * sub) * 128:
                                       (src_col + (i + 1) * sub) * 128],
                            remote_sem=rsem, local_sem=lsem,
                            rdests=rdests, queue_num=RQ).then_inc(psem, 1)
                    mks.append(mk)
                state["npr"] += len(slots)
                npr = state["npr"]
                nsl = len(slots)
                mks.append(lambda: nc.gpsimd.wait_ge(psem, npr))
                mks.append(lambda: nc.gpsimd.trigger_dma(count=nsl,
                                                         queue_num=RQ))
                return mks

            def notify(sem, decl, slot):
                def mk():
                    rdests = [None] * 8
                    rdests[slot] = (0, decl)
                    return nc.gpsimd.remote_sem_update_broadcast(
                        remote_sem=sem, local_sem=lsem,
                        rdests=rdests, queue_num=RQ).then_inc(psem, 1)
                state["npr"] += 1
                npr = state["npr"]
                return [mk,
                        lambda: nc.gpsimd.wait_ge(psem, npr),
                        lambda: nc.gpsimd.trigger_dma(count=1, queue_num=RQ)]

            # ---------------- phase 2: gather + segment matmul --------------
            with nc.named_scope("phase2"):
                ps_g = None
                flush_of_grp = {}
                for sch in range(nsc):
                    gt = gathp.tile([128, NQ * TPC, 128], BF16)
                    for q in range(NQ):
                        ch = sch * NQ + q
                        it = idxp.tile([128, CPC // 16], I16)
                        nc.sync.dma_start(it[:], idx_d[ch])
                        nc.gpsimd.dma_gather(
                            gt[:, q * TPC:(q + 1) * TPC, :], tab_d[:],
                            it[:], CPC, CPC, OUTF,
                            single_packet=False, queue_num=q)
                    for q in range(NQ):
                        ch = sch * NQ + q
                        oh3 = ohp.tile([128, TPC, 128], BF16)
                        nc.vector.tensor_tensor(
                            out=oh3[:],
                            in0=iota_t[:].rearrange("p (o f) -> p o f", o=1)
                                .to_broadcast([128, TPC, 128]),
                            in1=DSTL[:, ch * TPC:(ch + 1) * TPC].to_broadcast(
                                [128, TPC, 128]),
                            op=mybir.AluOpType.is_equal)
                        for t in range(TPC):
                            g = ch * TPC + t
                            col = int(tile_col[g])
                            if col < 0:
                                continue
                            oh = oh3[:, t, :]
                            gtt = gt[:, q * TPC + t, :]
                            u_in_col = g - int(col_base[col])
                            grp, ucol = col // 4, col % 4
                            first = (ucol == 0 and u_in_col == 0)
                            last = (ucol == 3 and u_in_col == int(t_col[col]) - 1)
                            if first:
                                ps_g = psp.tile([128, 512], F32, space="PSUM",
                                                tag="ps")
                            nc.tensor.matmul(
                                out=ps_g[:, ucol * 128:(ucol + 1) * 128],
                                lhsT=oh, rhs=gtt,
                                start=first, stop=last)
                            if last:
                                fl = nc.vector.tensor_copy(
                                    out=PART[:, grp * 512:(grp + 1) * 512],
                                    in_=ps_g[:])
                                flush_of_grp[grp] = fl
                                if grp in (59, 79):  # G0 / G1 blocks done
                                    # separate nop carries the esem update:
                                    # the flush itself already carries the
                                    # Tile-generated sync updates (walrus
                                    # limit).
                                    nop = nc.engines[
                                        mybir.EngineType.DVE].nop(nofuse=True)
                                    nop.then_inc(esem, 1)
                                    add_dep_helper(
                                        nop.ins, fl.ins, sync=True,
                                        reason="esem nop after flush")
            # ---------------- exchange tail (vector adds + S2 sends) --------
            # One critical section; per-engine emission order is preserved
            # inside the inner basic block, and the vector engine only
            # branches in after all phase-2 flushes have executed.
            with nc.named_scope("exchtail"):
              with tc.tile_critical(no_gpsimd_drain=True):
                nc.gpsimd.load_library(library_config.remote_dma)
                # all cores must be in the remote_dma library before ANY
                # remote traffic flies — without this rendezvous the
                # exchange stalls for milliseconds.
                nc.gpsimd.bir_kernel_barrier_wait([list(range(P))])
                # gpsimd: both far-half sends fire concurrently once their
                # source columns are flushed (esem) — 8 sends on slots 4-7.
                nc.gpsimd.wait_ge(esem, 2)
                for mk in far_send(160, RECV) + far_send(240, RECV2):
                    mk()
                # vector: fold far-half arrivals, then the direct g-adds
                nc.vector.wait_ge(rsem, 16)
                nc.vector.tensor_tensor(
                    out=PART[:, 80 * 128:160 * 128],
                    in0=PART[:, 80 * 128:160 * 128],
                    in1=RECV2[:, 0:80 * 128],
                    op=mybir.AluOpType.add).then_inc(xsem, 1)
                nc.vector.tensor_tensor(
                    out=PART[:, 0:80 * 128],
                    in0=PART[:, 0:80 * 128],
                    in1=RECV[:, 0:80 * 128],
                    op=mybir.AluOpType.add).then_inc(xsem, 1)

                # gpsimd: notify decl2/decl3 peers (RECV2 free), send g2/g3
                # direct to owners once their notifies arrive; then the same
                # for g1 via RECV after the r0 add.
                mks = [lambda: nc.gpsimd.wait_ge(xsem, 1)]
                mks += notify(n2sem, 2, 1)
                mks += notify(n3sem, 3, 2)
                mks += [lambda: nc.gpsimd.wait_ge(n2sem, 2)]
                mks += direct_send(80, 40, 2, [1], RECV2, 0)
                mks += [lambda: nc.gpsimd.wait_ge(n3sem, 2)]
                mks += direct_send(120, 40, 3, [2], RECV2, 40)
                mks += [lambda: nc.gpsimd.wait_ge(xsem, 2)]
                mks += notify(n1sem, 1, 0)
                mks += [lambda: nc.gpsimd.wait_ge(n1sem, 2)]
                mks += direct_send(40, 40, 1, [0], RECV, 0)
                for mk in mks:
                    mk()

                # final adds: fold g1/g2/g3 into own columns
                # (16 far incs + 3 direct sends x 2 incs)
                nc.vector.wait_ge(rsem, 22)
                for buf, off in ((RECV2, 0), (RECV2, 40), (RECV, 0)):
                    nc.vector.tensor_tensor(
                        out=PART[:, 0:40 * 128],
                        in0=PART[:, 0:40 * 128],
                        in1=buf[:, off * 128:(off + 40) * 128],
                        op=mybir.AluOpType.add)
                nc.gpsimd.wait_ge(lsem, 16 * state["npr"])
                nc.gpsimd.load_library(library_config.mlp)

            # ---------------- phase 3b: add reduced partials + store --------
            with nc.named_scope("phase3"):
                for grp in range(10):
                    ost = stagep.tile([128, 512], F32, tag="ost")
                    ad = nc.vector.tensor_tensor(
                        out=ost[:], in0=OUT5[:, grp * 512:(grp + 1) * 512],
                        in1=PART[:, grp * 512:(grp + 1) * 512],
                        op=mybir.AluOpType.add)
                    add_dep_helper(ad.ins, p3a_copies[grp].ins, sync=True,
                                   reason="p3b after out5 copy")
                    nc.sync.dma_start(
                        out_d[grp * 512:(grp + 1) * 512, :].rearrange(
                            "(u p) f -> p u f", p=128),
                        ost[:].rearrange("p (u f) -> p u f", u=4))
    nc.compile()
    return nc


# ============================ public entry ================================

def _install_ntff_hook():
    """The agent image lacks antenv.axon_hooks; recreate it and register the
    ctypes NTFF profile hook so trace=True works under axon."""
    import types
    import antenv
    if "antenv.axon_hooks" not in sys.modules:
        m = types.ModuleType("antenv.axon_hooks")
        _h = [None]
        m.get_axon_ntff_profile_hook = lambda: _h[0]
        m.set_axon_ntff_profile_hook = lambda h: _h.__setitem__(0, h)
        sys.modules["antenv.axon_hooks"] = m
        antenv.axon_hooks = m
    import antenv.axon_hooks as ah
    if ah.get_axon_ntff_profile_hook() is None:
        try:
            from trn_agent_boot.trn_boot import _ntff_profile_via_ctypes
            ah.set_axon_ntff_profile_hook(
                _ntff_profile_via_ctypes("/opt/axon/libaxon_pjrt.so"))
        except Exception as e:
            print(f"ntff hook install failed ({e}); timing disabled")


def kernel(x, W1, b1, W2, b2, src_ids, dst_ids, n_dst):
    n_dst = int(n_dst)
    assert n_dst == N_DST
    in_maps, sched = _host_prep(x, W1, b1, W2, b2, src_ids, dst_ids)
    key = (sched["nchunk"], tuple(sched["t_col"].tolist()))
    if key not in _CACHE:
        _CACHE.clear()
        _CACHE[key] = _build(sched)
    nc = _CACHE[key]
    trace = bool(os.environ.get("BASS_KERNEL_TRACE"))
    kw = {}
    if trace:
        _install_ntff_hook()
        tcores = [0]
        if os.environ.get("TRACE_ALL_CORES"):
            tcores = list(range(P))
        kw = dict(trace=True, trace_cores=tcores, stitch_traces=False)
    res = run_bass_kernel_spmd(nc, in_maps, core_ids=list(range(P)), **kw)
    if trace:
        print(f"HW exec time: {res.exec_time_ns} ns")
        if res.per_core_scope_times:
            for scope, m in sorted(res.per_core_scope_times.items()):
                print(f"  scope {scope}: {m}")
        if res.instructions_and_trace:
            print(f"  trace: {res.instructions_and_trace[1]}")
    out = np.concatenate([res.results[c]["out"] for c in range(P)], axis=0)
    return np.ascontiguousarray(out[:N_DST]).astype(np.float32)


if __name__ == "__main__":
    # smoke test with random data
    rng = np.random.default_rng(0)
    x = rng.standard_normal((N_SRC, INF), dtype=np.float32)
    W1 = rng.standard_normal((OUTF, INF), dtype=np.float32) * 0.0625
    W2 = rng.standard_normal((OUTF, INF), dtype=np.float32) * 0.0625
    b1 = np.zeros(OUTF, np.float32)
    b2 = np.zeros(OUTF, np.float32)
    src = rng.integers(0, N_SRC, N_EDGES).astype(np.int32)
    dst = np.sort(rng.integers(0, N_DST, N_EDGES).astype(np.int32))
    got = kernel(x, W1, b1, W2, b2, src, dst, N_DST)
    proj = x @ W1.T + b1
    want = np.zeros((N_DST, OUTF), np.float32)
    np.add.at(want, dst, proj[src])
    want += x[:N_DST] @ W2.T + b2
    denom = np.abs(want).max()
    print("rel err:", np.abs(got - want).max() / denom)


# revision 42
# speedup vs baseline: 1.0559x; 1.0559x over previous
"""DistSageConv on 8 TRN2 NeuronCores (Bass/Tile) — v2 overlapped.

Reference computation:
    out  = x @ W1.T + b1                                  # [n_src, 128]
    out1 = segment_sum(out[src_ids], dst_ids, n_dst)      # [n_dst, 128]
    out5 = x[:n_dst] @ W2.T + b2
    return out5 + out1

Distribution: src nodes sharded across 8 cores; each core projects its own
src shard (bf16) into a local bf16 table in DRAM, gathers its own-src edges'
rows with SWDGE dma_gather, segment-reduces them with one-hot matmuls on the
PE (dst grouped into 128-row blocks, XOR-permuted owner-major column order),
then partial aggregates are exchanged in one critical section after phase 2:

  - a cross-core gpsimd barrier first ensures every core has the remote_dma
    library loaded (remote traffic before that rendezvous stalls for
    MILLISECONDS — this cost a lot of debugging);
  - both far-half blocks (320 local columns' groups 4-7) are sent to the
    decl-6 (cross-die D2D) peer CONCURRENTLY as 8 x 20-column sends into
    RECV/RECV2 — unlike the baseline's 4 serial recursive-halving rounds;
  - the on-die remainder is exchanged with three DIRECT per-owner sends
    (decl 1/2/3 land owner groups 1/2/3 on their owners), gated by notify
    semaphores that let peers reuse RECV/RECV2 once the far-half adds have
    consumed them.  Notify hops are ~1 us after the barrier fix.

Columns are processed in far-groups-first order so the far-half flushes
complete earliest (esem), letting the far sends fire as soon as the gpsimd
queue drains.  Owners fuse x[:n_dst] @ W2.T (+bias, +deg*b1 via an augmented
K=258 matmul) and write their 5120-row output slab.  All x-side inputs are
fed in bf16 (PSUM accumulation stays f32; rel err ~7e-3 vs the f32
reference, unchanged from the all-f32-input baseline).

Baseline (4-round serial exchange + f32 phase 1): 724 us.  This version:
~590 us (phase1 bf16 ~65 us, phase2 gather+matmul ~375 us, exchange tail +
output ~150 us).
"""
import sys
sys.path.insert(0, "/opt/trn_rl_repo")

import numpy as np
import ml_dtypes

import os
import concourse.bacc as bacc
import concourse.bass as bass
import concourse.mybir as mybir
import concourse.tile as tile
from concourse import library_config
from concourse.bass_utils import run_bass_kernel_spmd
from concourse.tile import add_dep_helper

# ---------------- problem constants (hardcoded per contract) --------------
P = 8                      # cores
N_SRC = 100000
N_DST = 40000
N_EDGES = 640000
INF = 256                  # in_feats
OUTF = 128                 # out_feats
SRC_SH = N_SRC // P        # 12500 src rows per core
SRC_PAD = 12800            # padded table rows (25 x 512)
NBLK = 320                 # padded dst blocks of 128 (40960 dst rows)
BPC = NBLK // P            # 40 blocks (columns) owned per core
DST_PAD = NBLK * 128       # 40960
CPC = 2048                 # gather chunk edges
TPC = CPC // 128           # 16 tiles per chunk
NQ = 4                     # SWDGE queues for gathers (round-robin, overlap)
RQ = 0                     # SWDGE queue for remote sends/notifies (all fired
                           # after the gathers have fully drained, as in the
                           # baseline: mid-phase2 remote sends stall for
                           # milliseconds — the interleaved library reloads
                           # appear to wreck in-flight SWDGE ring state)

F32 = mybir.dt.float32
BF16 = mybir.dt.bfloat16
I16 = mybir.dt.int16

# column processing order: far-half groups first, own group (0) last
COL_ORDER = (list(range(160, 240)) + list(range(240, 320)) +
             list(range(80, 160)) + list(range(40, 80)) + list(range(0, 40)))

_CACHE = {}


# ============================ host-side prep ==============================

def _wrap_idxs(idx):
    """[n] int16 -> [128, n//16] wrapped in 16 partitions, replicated x8."""
    n = len(idx)
    w = np.zeros((128, n // 16), dtype=np.int16)
    for p in range(16):
        w[p, :] = idx[p::16]
    for r in range(1, 8):
        w[16 * r:16 * r + 16, :] = w[:16, :]
    return w


def _host_prep(x, W1, b1, W2, b2, src_ids, dst_ids):
    """Build per-core input arrays + the static tile->column schedule."""
    x = np.asarray(x, np.float32)
    W1 = np.asarray(W1, np.float32)
    W2 = np.asarray(W2, np.float32)
    b1 = np.asarray(b1, np.float32).reshape(-1)
    b2 = np.asarray(b2, np.float32).reshape(-1)
    src_ids = np.asarray(src_ids, np.int64)
    dst_ids = np.asarray(dst_ids, np.int64)

    owner = src_ids // SRC_SH                       # edge -> src-owner core
    blk = dst_ids // 128                            # edge -> dst block
    deg_full = np.bincount(dst_ids, minlength=DST_PAD).astype(np.float32)

    # processing position of each column
    pos_of_col = np.zeros(NBLK, dtype=np.int64)
    for i, j in enumerate(COL_ORDER):
        pos_of_col[j] = i

    # per-(core, column) counts;  column j on core c holds block beta_c(j)
    per_core = []
    for c in range(P):
        m = owner == c
        e_src = (src_ids[m] - c * SRC_SH).astype(np.int64)
        e_dst = dst_ids[m]
        e_blk = blk[m]
        col = ((e_blk // BPC) ^ c) * BPC + (e_blk % BPC)
        order = np.argsort(pos_of_col[col], kind="stable")
        per_core.append((e_src[order], e_dst[order], col[order]))

    counts = np.zeros((P, NBLK), dtype=np.int64)
    for c in range(P):
        counts[c] = np.bincount(per_core[c][2], minlength=NBLK)
    t_col = np.maximum(1, (counts.max(axis=0) + 127) // 128)   # tiles per col
    nt_tot = int(t_col.sum())
    spc = NQ * TPC             # tiles per super-chunk (NQ queues bunched)
    nt_pad = ((nt_tot + spc - 1) // spc) * spc
    nchunk = nt_pad // TPC
    # col_base in PROCESSING order
    col_base = np.zeros(NBLK + 1, dtype=np.int64)
    acc = 0
    for j in COL_ORDER:
        col_base[j] = acc
        acc += t_col[j]

    # static schedule: per tile -> col ;  col -1 => pad tile
    tile_col = np.full(nt_pad, -1, dtype=np.int64)
    for j in range(NBLK):
        tile_col[col_base[j]:col_base[j] + t_col[j]] = j

    # stream positions where the exchange G-blocks complete
    g0_end = int(sum(t_col[j] for j in range(160, 240)))
    g1_end = g0_end + int(sum(t_col[j] for j in range(240, 320)))

    in_maps = []
    iota = np.broadcast_to(np.arange(128, dtype=np.float32), (128, 128))
    iota = np.ascontiguousarray(iota.astype(ml_dtypes.bfloat16))
    W1T = np.ascontiguousarray(W1.T.astype(ml_dtypes.bfloat16))   # [256, 128]
    W2T_aug = np.concatenate([W2.T, b1[None, :], b2[None, :]], axis=0)
    W2T_aug = np.ascontiguousarray(W2T_aug.astype(ml_dtypes.bfloat16))

    for c in range(P):
        e_src, e_dst, e_col = per_core[c]
        # scatter edges into padded per-column tile runs
        # pad slots must NOT all hit one table row (HBM hot-bank) —
        # spread them across the table; dst_arr=-1 keeps onehot columns zero.
        idx_arr = (np.arange(nt_pad * 128) % SRC_PAD).astype(np.int16)
        dst_arr = np.full(nt_pad * 128, -1.0, dtype=np.float32)
        cc = np.bincount(e_col, minlength=NBLK)
        cstart = np.zeros(NBLK + 1, dtype=np.int64)
        cstart[1:] = np.cumsum(cc)
        # position of each edge inside its column run (edges sorted by
        # processing position, stable → within col they are consecutive)
        pos_in_col = np.zeros(len(e_col), dtype=np.int64)
        run_start = np.zeros(NBLK, dtype=np.int64)
        # edges of column j occupy a consecutive run; compute run starts in
        # processing order
        acc2 = 0
        for j in COL_ORDER:
            run_start[j] = acc2
            acc2 += cc[j]
        pos_in_col = np.arange(len(e_col)) - run_start[e_col]
        gpos = col_base[e_col] * 128 + pos_in_col
        idx_arr[gpos] = e_src.astype(np.int16)
        e_blk_of_col = ((e_col // BPC) ^ c) * BPC + (e_col % BPC)
        dst_arr[gpos] = (e_dst - e_blk_of_col * 128).astype(np.float32)

        idx_dram = np.zeros((nchunk, 128, CPC // 16), dtype=np.int16)
        for ch in range(nchunk):
            idx_dram[ch] = _wrap_idxs(idx_arr[ch * CPC:(ch + 1) * CPC])
        dst_dram = dst_arr.reshape(nchunk, TPC, 128).transpose(2, 0, 1)
        dst_dram = np.ascontiguousarray(
            dst_dram.reshape(128, nchunk * TPC).astype(ml_dtypes.bfloat16))

        xT = np.zeros((INF, SRC_PAD), dtype=ml_dtypes.bfloat16)
        xT[:, :SRC_SH] = x[c * SRC_SH:(c + 1) * SRC_SH].T.astype(
            ml_dtypes.bfloat16)
        lo, hi = c * 5120, min((c + 1) * 5120, N_DST)
        xdT = np.zeros((INF + 2, 5120), dtype=np.float32)
        if hi > lo:
            xdT[:INF, :hi - lo] = x[lo:hi].T
        xdT[INF, :] = deg_full[c * 5120:(c + 1) * 5120]
        xdT[INF + 1, :] = 1.0
        xdT = xdT.astype(ml_dtypes.bfloat16)

        in_maps.append({
            "xT": np.ascontiguousarray(xT),
            "xdT": np.ascontiguousarray(xdT),
            "W1T": W1T,
            "W2Ta": W2T_aug,
            "iota": iota,
            "idx": idx_dram,
            "dstloc": dst_dram,
        })

    sched = {"t_col": t_col, "nchunk": nchunk, "tile_col": tile_col,
             "col_base": col_base, "g0_end": g0_end, "g1_end": g1_end}
    return in_maps, sched


# ============================ device program ==============================

def _build(sched):
    t_col = sched["t_col"]
    nchunk = sched["nchunk"]
    tile_col = sched["tile_col"]
    col_base = sched["col_base"]
    nt_pad = nchunk * TPC
    nsc = nchunk // NQ          # super-chunks

    # super-chunk indices after which the R0 / R1 remote sections run.
    # Desc-gen is consumption-throttled (gather pool depth), so it lags the
    # flushes; fire the sends shortly after the G-block tiles are issued.
    LEAD = NQ * TPC
    sc_r0 = min(nsc - 1, (sched["g0_end"] + LEAD) // (NQ * TPC) + 1)
    sc_r1 = min(nsc - 1, max(sc_r0 + 1,
                             (sched["g1_end"] + LEAD) // (NQ * TPC) + 1))

    nc = bacc.Bacc("TRN2", target_bir_lowering=False, debug=False,
                   num_devices=P, num_swdge_queues=NQ)

    xT_d = nc.dram_tensor("xT", [INF, SRC_PAD], BF16, kind="ExternalInput")
    xdT_d = nc.dram_tensor("xdT", [INF + 2, 5120], BF16, kind="ExternalInput")
    W1T_d = nc.dram_tensor("W1T", [INF, OUTF], BF16, kind="ExternalInput")
    W2Ta_d = nc.dram_tensor("W2Ta", [INF + 2, OUTF], BF16, kind="ExternalInput")
    iota_d = nc.dram_tensor("iota", [128, 128], BF16, kind="ExternalInput")
    idx_d = nc.dram_tensor("idx", [nchunk, 128, CPC // 16], I16, kind="ExternalInput")
    dst_d = nc.dram_tensor("dstloc", [128, nchunk * TPC], BF16, kind="ExternalInput")
    out_d = nc.dram_tensor("out", [5120, OUTF], F32, kind="ExternalOutput")
    tab_d = nc.dram_tensor("tab", [SRC_PAD, OUTF], BF16, kind="Internal")

    # persistent SBUF: partial slab + exchange recv buffers
    PART = nc.alloc_sbuf_tensor("part", [128, NBLK * 128], BF16)
    RECV = nc.alloc_sbuf_tensor("recv", [128, (NBLK // 4) * 128], BF16)
    RECV2 = nc.alloc_sbuf_tensor("recv2", [128, (NBLK // 4) * 128], BF16)
    OUT5 = nc.alloc_sbuf_tensor("out5", [128, BPC * 128], BF16)

    rsem = nc.semaphore("rsem").__enter__()      # data arrived (remote inc)
    n1sem = nc.semaphore("n1sem").__enter__()    # notify: peer freed RECV low
    n2sem = nc.semaphore("n2sem").__enter__()    # notify: peer freed RECV2 lo
    n3sem = nc.semaphore("n3sem").__enter__()    # notify: peer freed RECV2 hi
    lsem = nc.semaphore("lsem").__enter__()      # local send drained
    psem = nc.semaphore("psem").__enter__()      # desc-gen complete
    esem = nc.semaphore("esem").__enter__()      # G-block flushes done
    xsem = nc.semaphore("xsem").__enter__()      # DVE add done

    # bookkeeping for manual gpsimd sections
    state = {"npr": 0}

    with tile.TileContext(nc) as tc:
        nc.gpsimd.load_library(library_config.mlp)
        with (
            tc.tile_pool(name="consts", bufs=1) as constp,
            tc.tile_pool(name="xab", bufs=2) as xabp,
            tc.tile_pool(name="xc2", bufs=2) as xc2p,
            tc.tile_pool(name="stage", bufs=2) as stagep,
            tc.tile_pool(name="idx", bufs=8) as idxp,
            tc.tile_pool(name="gath", bufs=3) as gathp,
            tc.tile_pool(name="oh", bufs=2) as ohp,
            tc.tile_pool(name="ps", bufs=4, space="PSUM") as psp,
            tc.tile_pool(name="ps3", bufs=2, space="PSUM") as ps3p,
        ):
            # ---- constants
            iota_t = constp.tile([128, 128], BF16)
            nc.sync.dma_start(iota_t[:], iota_d[:])
            w1 = constp.tile([128, 2, OUTF], BF16)
            nc.sync.dma_start(w1[:], W1T_d[:].rearrange("(k p) f -> p k f", p=128))
            w2 = constp.tile([128, 2, OUTF], BF16)
            nc.sync.dma_start(w2[:], W2Ta_d[:INF].rearrange("(k p) f -> p k f", p=128))
            wb = constp.tile([2, OUTF], BF16)
            nc.sync.dma_start(wb[:], W2Ta_d[INF:INF + 2, :])
            DSTL = constp.tile([128, nchunk * TPC], BF16)
            nc.sync.dma_start(DSTL[:], dst_d[:])

            # ---------------- phase 1: project own src shard ----------------
            with nc.named_scope("phase1"):
                for j in range(SRC_PAD // 512):
                    a0 = xabp.tile([128, 512], BF16, tag="a0")
                    a1 = xabp.tile([128, 512], BF16, tag="a1")
                    nc.sync.dma_start(a0[:], xT_d[0:128, j * 512:(j + 1) * 512])
                    nc.sync.dma_start(a1[:], xT_d[128:256, j * 512:(j + 1) * 512])
                    ps = psp.tile([128, 512], F32, space="PSUM", tag="ps")
                    for u in range(4):
                        nc.tensor.matmul(
                            out=ps[:, u * 128:(u + 1) * 128],
                            lhsT=a0[:, u * 128:(u + 1) * 128], rhs=w1[:, 0, :],
                            start=(u == 0), stop=False)
                        nc.tensor.matmul(
                            out=ps[:, u * 128:(u + 1) * 128],
                            lhsT=a1[:, u * 128:(u + 1) * 128], rhs=w1[:, 1, :],
                            start=False, stop=(u == 3))
                    st = stagep.tile([128, 512], BF16, tag="st1")
                    nc.vector.tensor_copy(out=st[:], in_=ps[:])
                    nc.sync.dma_start(
                        tab_d[j * 512:(j + 1) * 512, :].rearrange(
                            "(u p) f -> p u f", p=128),
                        st[:].rearrange("p (u f) -> p u f", u=4))

            # -------- phase 3a: own-dst projection into OUT5 (overlaps p2) --
            with nc.named_scope("phase3a"):
                p3a_copies = []
                for grp in range(10):
                    b0 = xc2p.tile([128, 512], BF16, tag="b0")
                    b1t = xc2p.tile([128, 512], BF16, tag="b1")
                    b2t = xc2p.tile([2, 512], BF16, tag="b2")
                    nc.sync.dma_start(b0[:], xdT_d[0:128, grp * 512:(grp + 1) * 512])
                    nc.sync.dma_start(b1t[:], xdT_d[128:256, grp * 512:(grp + 1) * 512])
                    nc.sync.dma_start(b2t[:], xdT_d[256:258, grp * 512:(grp + 1) * 512])
                    ps3 = ps3p.tile([128, 512], F32, space="PSUM", tag="p3")
                    for u in range(4):
                        sl = slice(u * 128, (u + 1) * 128)
                        nc.tensor.matmul(out=ps3[:, sl], lhsT=b0[:, sl],
                                         rhs=w2[:, 0, :], start=(u == 0), stop=False)
                        nc.tensor.matmul(out=ps3[:, sl], lhsT=b1t[:, sl],
                                         rhs=w2[:, 1, :], start=False, stop=False)
                        nc.tensor.matmul(out=ps3[:, sl], lhsT=b2t[:, sl],
                                         rhs=wb[:], start=False, stop=(u == 3))
                    cp = nc.scalar.copy(
                        out=OUT5[:, grp * 512:(grp + 1) * 512], in_=ps3[:])
                    p3a_copies.append(cp)

            # ----- manual remote helpers (critical sections) ----------------
            # Remote desc-gen ops + their sem waits live inside
            # tc.tile_critical() bodies: the scheduler treats the inner basic
            # block as opaque, so the cross-core semaphore waits don't trip
            # the (single-core) deadlock checker, and per-engine emission
            # order inside the block is preserved on hardware.
            def remote_section(body):
                """Switch to remote_dma lib, run body(), switch back to mlp,
                inside one critical section."""
                with tc.tile_critical(no_gpsimd_drain=True):
                    nc.gpsimd.load_library(library_config.remote_dma)
                    for mk in body:
                        mk()
                    nc.gpsimd.load_library(library_config.mlp)

            def far_send(src_col, recv_buf):
                """4 sends of 20 cols to decl-6 peer; slots 4-7."""
                mks = []
                for i in range(4):
                    def mk(i=i):
                        rdests = [None] * 8
                        rdests[4 + i] = (0, 6)
                        return nc.gpsimd.remote_dma_broadcast(
                            out_ap=recv_buf[:, i * 20 * 128:(i + 1) * 20 * 128],
                            in_ap=PART[:, (src_col + i * 20) * 128:
                                       (src_col + (i + 1) * 20) * 128],
                            remote_sem=rsem, local_sem=lsem,
                            rdests=rdests, queue_num=RQ).then_inc(psem, 1)
                    mks.append(mk)
                state["npr"] += 4
                npr = state["npr"]
                mks.append(lambda: nc.gpsimd.wait_ge(psem, npr))
                mks.append(lambda: nc.gpsimd.trigger_dma(count=4,
                                                         queue_num=RQ))
                return mks

            def direct_send(src_col, ncols, decl, slots, recv_buf, recv_off):
                """Send split across len(slots) engine pairs."""
                sub = ncols // len(slots)
                mks = []
                for i, slot in enumerate(slots):
                    def mk(i=i, slot=slot):
                        rdests = [None] * 8
                        rdests[slot] = (0, decl)
                        return nc.gpsimd.remote_dma_broadcast(
                            out_ap=recv_buf[:, (recv_off + i * sub) * 128:
                                            (recv_off + (i + 1) * sub) * 128],
                            in_ap=PART[:, (src_col + i * sub) * 128:
                                       (src_col + (i + 1) * sub) * 128],
                            remote_sem=rsem, local_sem=lsem,
                            rdests=rdests, queue_num=RQ).then_inc(psem, 1)
                    mks.append(mk)
                state["npr"] += len(slots)
                npr = state["npr"]
                nsl = len(slots)
                mks.append(lambda: nc.gpsimd.wait_ge(psem, npr))
                mks.append(lambda: nc.gpsimd.trigger_dma(count=nsl,
                                                         queue_num=RQ))
                return mks

            def notify(sem, decl, slot):
                def mk():
                    rdests = [None] * 8
                    rdests[slot] = (0, decl)
                    return nc.gpsimd.remote_sem_update_broadcast(
                        remote_sem=sem, local_sem=lsem,
                        rdests=rdests, queue_num=RQ).then_inc(psem, 1)
                state["npr"] += 1
                npr = state["npr"]
                return [mk,
                        lambda: nc.gpsimd.wait_ge(psem, npr),
                        lambda: nc.gpsimd.trigger_dma(count=1, queue_num=RQ)]

            # ---------------- phase 2: gather + segment matmul --------------
            with nc.named_scope("phase2"):
                ps_g = None
                flush_of_grp = {}
                # fire the far sends once G1's flushes are certainly close:
                # desc-gen runs ~2-3 super-chunks ahead of the matmul/flush
                # pipeline (gather pool depth).
                sc_fire = min(nsc - 2,
                              (sched["g1_end"] + 2 * NQ * TPC)
                              // (NQ * TPC) + 1)
                for sch in range(nsc):
                    gt = gathp.tile([128, NQ * TPC, 128], BF16)
                    for q in range(NQ):
                        ch = sch * NQ + q
                        it = idxp.tile([128, CPC // 16], I16)
                        nc.sync.dma_start(it[:], idx_d[ch])
                        nc.gpsimd.dma_gather(
                            gt[:, q * TPC:(q + 1) * TPC, :], tab_d[:],
                            it[:], CPC, CPC, OUTF,
                            single_packet=False, queue_num=q)
                    for q in range(NQ):
                        ch = sch * NQ + q
                        oh3 = ohp.tile([128, TPC, 128], BF16)
                        nc.vector.tensor_tensor(
                            out=oh3[:],
                            in0=iota_t[:].rearrange("p (o f) -> p o f", o=1)
                                .to_broadcast([128, TPC, 128]),
                            in1=DSTL[:, ch * TPC:(ch + 1) * TPC].to_broadcast(
                                [128, TPC, 128]),
                            op=mybir.AluOpType.is_equal)
                        for t in range(TPC):
                            g = ch * TPC + t
                            col = int(tile_col[g])
                            if col < 0:
                                continue
                            oh = oh3[:, t, :]
                            gtt = gt[:, q * TPC + t, :]
                            u_in_col = g - int(col_base[col])
                            grp, ucol = col // 4, col % 4
                            first = (ucol == 0 and u_in_col == 0)
                            last = (ucol == 3 and u_in_col == int(t_col[col]) - 1)
                            if first:
                                ps_g = psp.tile([128, 512], F32, space="PSUM",
                                                tag="ps")
                            nc.tensor.matmul(
                                out=ps_g[:, ucol * 128:(ucol + 1) * 128],
                                lhsT=oh, rhs=gtt,
                                start=first, stop=last)
                            if last:
                                fl = nc.vector.tensor_copy(
                                    out=PART[:, grp * 512:(grp + 1) * 512],
                                    in_=ps_g[:])
                                flush_of_grp[grp] = fl
                                if grp in (59, 79):  # G0 / G1 far blocks done
                                    # separate nop carries the esem update:
                                    # the flush itself already carries the
                                    # Tile-generated sync updates (walrus
                                    # limit).
                                    nop = nc.engines[
                                        mybir.EngineType.DVE].nop(nofuse=True)
                                    nop.then_inc(esem, 1)
                                    add_dep_helper(
                                        nop.ins, fl.ins, sync=True,
                                        reason="esem nop after flush")
                    if sch == sc_fire:
                        # fire BOTH far-half sends mid-phase-2: the barrier
                        # rendezvous (all cores in remote_dma lib) happens
                        # here; the D2D wire then hides under the remaining
                        # G2/G3 gather work.
                        with nc.named_scope("exchfar"):
                          with tc.tile_critical(no_gpsimd_drain=True):
                            nc.gpsimd.load_library(library_config.remote_dma)
                            nc.gpsimd.bir_kernel_barrier_wait(
                                [list(range(P))])
                            nc.gpsimd.wait_ge(esem, 2)
                            for mk in (far_send(160, RECV)
                                       + far_send(240, RECV2)):
                                mk()
                            nc.gpsimd.load_library(library_config.mlp)
            # ---------------- exchange tail (vector adds + S2 sends) --------
            # One critical section; per-engine emission order is preserved
            # inside the inner basic block, and the vector engine only
            # branches in after all phase-2 flushes have executed.
            with nc.named_scope("exchtail"):
              with tc.tile_critical(no_gpsimd_drain=True):
                nc.gpsimd.load_library(library_config.remote_dma)
                # vector: fold far-half arrivals, then the direct g-adds
                nc.vector.wait_ge(rsem, 16)
                nc.vector.tensor_tensor(
                    out=PART[:, 80 * 128:160 * 128],
                    in0=PART[:, 80 * 128:160 * 128],
                    in1=RECV2[:, 0:80 * 128],
                    op=mybir.AluOpType.add).then_inc(xsem, 1)
                nc.vector.tensor_tensor(
                    out=PART[:, 0:80 * 128],
                    in0=PART[:, 0:80 * 128],
                    in1=RECV[:, 0:80 * 128],
                    op=mybir.AluOpType.add).then_inc(xsem, 1)

                # gpsimd: notify decl2/decl3 peers (RECV2 free), send g2/g3
                # direct to owners once their notifies arrive; then the same
                # for g1 via RECV after the r0 add.
                mks = [lambda: nc.gpsimd.wait_ge(xsem, 1)]
                mks += notify(n2sem, 2, 1)
                mks += notify(n3sem, 3, 2)
                mks += [lambda: nc.gpsimd.wait_ge(n2sem, 2)]
                mks += direct_send(80, 40, 2, [1], RECV2, 0)
                mks += [lambda: nc.gpsimd.wait_ge(n3sem, 2)]
                mks += direct_send(120, 40, 3, [2], RECV2, 40)
                mks += [lambda: nc.gpsimd.wait_ge(xsem, 2)]
                mks += notify(n1sem, 1, 0)
                mks += [lambda: nc.gpsimd.wait_ge(n1sem, 2)]
                mks += direct_send(40, 40, 1, [0], RECV, 0)
                for mk in mks:
                    mk()

                # final adds: fold g1/g2/g3 into own columns
                # (16 far incs + 3 direct sends x 2 incs)
                nc.vector.wait_ge(rsem, 22)
                for buf, off in ((RECV2, 0), (RECV2, 40), (RECV, 0)):
                    nc.vector.tensor_tensor(
                        out=PART[:, 0:40 * 128],
                        in0=PART[:, 0:40 * 128],
                        in1=buf[:, off * 128:(off + 40) * 128],
                        op=mybir.AluOpType.add)
                nc.gpsimd.wait_ge(lsem, 16 * state["npr"])
                nc.gpsimd.load_library(library_config.mlp)

            # ---------------- phase 3b: add reduced partials + store --------
            with nc.named_scope("phase3"):
                for grp in range(10):
                    ost = stagep.tile([128, 512], F32, tag="ost")
                    ad = nc.vector.tensor_tensor(
                        out=ost[:], in0=OUT5[:, grp * 512:(grp + 1) * 512],
                        in1=PART[:, grp * 512:(grp + 1) * 512],
                        op=mybir.AluOpType.add)
                    add_dep_helper(ad.ins, p3a_copies[grp].ins, sync=True,
                                   reason="p3b after out5 copy")
                    nc.sync.dma_start(
                        out_d[grp * 512:(grp + 1) * 512, :].rearrange(
                            "(u p) f -> p u f", p=128),
                        ost[:].rearrange("p (u f) -> p u f", u=4))
    nc.compile()
    return nc


# ============================ public entry ================================

def _install_ntff_hook():
    """The agent image lacks antenv.axon_hooks; recreate it and register the
    ctypes NTFF profile hook so trace=True works under axon."""
    import types
    import antenv
    if "antenv.axon_hooks" not in sys.modules:
        m = types.ModuleType("antenv.axon_hooks")
        _h = [None]
        m.get_axon_ntff_profile_hook = lambda: _h[0]
        m.set_axon_ntff_profile_hook = lambda h: _h.__setitem__(0, h)
        sys.modules["antenv.axon_hooks"] = m
        antenv.axon_hooks = m
    import antenv.axon_hooks as ah
    if ah.get_axon_ntff_profile_hook() is None:
        try:
            from trn_agent_boot.trn_boot import _ntff_profile_via_ctypes
            ah.set_axon_ntff_profile_hook(
                _ntff_profile_via_ctypes("/opt/axon/libaxon_pjrt.so"))
        except Exception as e:
            print(f"ntff hook install failed ({e}); timing disabled")


def kernel(x, W1, b1, W2, b2, src_ids, dst_ids, n_dst):
    n_dst = int(n_dst)
    assert n_dst == N_DST
    in_maps, sched = _host_prep(x, W1, b1, W2, b2, src_ids, dst_ids)
    key = (sched["nchunk"], tuple(sched["t_col"].tolist()))
    if key not in _CACHE:
        _CACHE.clear()
        _CACHE[key] = _build(sched)
    nc = _CACHE[key]
    trace = bool(os.environ.get("BASS_KERNEL_TRACE"))
    kw = {}
    if trace:
        _install_ntff_hook()
        tcores = [0]
        if os.environ.get("TRACE_ALL_CORES"):
            tcores = list(range(P))
        kw = dict(trace=True, trace_cores=tcores, stitch_traces=False)
    res = run_bass_kernel_spmd(nc, in_maps, core_ids=list(range(P)), **kw)
    if trace:
        print(f"HW exec time: {res.exec_time_ns} ns")
        if res.per_core_scope_times:
            for scope, m in sorted(res.per_core_scope_times.items()):
                print(f"  scope {scope}: {m}")
        if res.instructions_and_trace:
            print(f"  trace: {res.instructions_and_trace[1]}")
    out = np.concatenate([res.results[c]["out"] for c in range(P)], axis=0)
    return np.ascontiguousarray(out[:N_DST]).astype(np.float32)


if __name__ == "__main__":
    # smoke test with random data
    rng = np.random.default_rng(0)
    x = rng.standard_normal((N_SRC, INF), dtype=np.float32)
    W1 = rng.standard_normal((OUTF, INF), dtype=np.float32) * 0.0625
    W2 = rng.standard_normal((OUTF, INF), dtype=np.float32) * 0.0625
    b1 = np.zeros(OUTF, np.float32)
    b2 = np.zeros(OUTF, np.float32)
    src = rng.integers(0, N_SRC, N_EDGES).astype(np.int32)
    dst = np.sort(rng.integers(0, N_DST, N_EDGES).astype(np.int32))
    got = kernel(x, W1, b1, W2, b2, src, dst, N_DST)
    proj = x @ W1.T + b1
    want = np.zeros((N_DST, OUTF), np.float32)
    np.add.at(want, dst, proj[src])
    want += x[:N_DST] @ W2.T + b2
    denom = np.abs(want).max()
    print("rel err:", np.abs(got - want).max() / denom)
